# revision 24
# baseline (speedup 1.0000x reference)
"""Trainium2 Bass kernel for GQA sliding-window causal attention.

Problem: B=2, S=2048, H=32 q-heads, KVH=8 kv-heads, D=128,
sliding window 1024, causal, scale 1/sqrt(128). f32 I/O.

Sharding (8 cores, pure tensor parallel, no collectives): core c gets
kv-head c and its query-head group [4c, 4c+4). Each core computes full
attention for its 4 q-heads over both batch elements; host concatenates
along the head dim.

Per-core algorithm (banded, no online softmax needed since scores are
O(1) and exp never overflows):
  - Q and K live in SBUF transposed: [d=128 partitions, s free].
  - Scores computed transposed, ST[k, q], per (k-tile, q-span) band
    segment. Segments are packed into NINE uniform [128, 1536] PSUM
    groups per (b, h) (3 banks each, 27 exactly-full 512-col bins, one
    group straddling the g0/g1 q-block boundary) so ONE Exp activation
    covers 1536 contiguous columns -> 9 activations per (b,h), 72 per
    core. The scalar (Activation) engine is the hard floor of this
    decomposition (1 col/cycle @ 1.2 GHz, 110592 cols/core = 92 us);
    every activation carries ~300 ns of fixed cost (PSUM access init +
    decode) and its exec queue depth is 0, so fewer, wider activations
    win. The scalar queue issues NO DMAs (a DMA trigger costs the
    issuing sequencer ~670 ns).
  - P = exp(SCALE * ST) on ScalarE (scale folded into the activation),
    written as bf16 to SBUF.
  - Causal-diagonal and window-edge tiles are masked AFTER exp by
    multiplying with 0/1 bf16 mask tiles on GpSimd (exact zeros).
    GpSimd is otherwise idle; DVE keeps only the finalizers.
  - PV: out[q, 0:129] += PT_slice.T @ V'_j where V' has a ones column
    appended -> col 128 accumulates the softmax denominator for free.
    TWO [128, 129] accumulators share each PSUM bank as ONE hardware
    accumulation group: start_tensor_calc on the bank's first matmul
    marks the whole 2 KB zero region pending-zero (per-address
    zero-or-accumulate on write), so the sibling accumulator's first
    start=False write still lands on logical zeros. This halves the
    accumulator bank footprint (2 banks for 4 accumulators), buying
    3-bank score groups: PSUM = 2 score groups (6 banks) + 2 banks.
  - Normalize: out = psum[:, :128] * reciprocal(psum[:, 128]) on DVE,
    into a per-q-block [128, 512] staging tile flushed by ONE batched
    out-DMA on the sync queue.
  - 3-stage flat software pipeline across ALL (b, h, group) work:
    QK(w) | exp+mask(w-1) | PV+finalize(w-2), so the PE always has
    independent work while ScalarE runs and no pipeline drain at
    block/head/batch boundaries.
  - Input DMAs are chunked; K/V ride the sync HWDGE queue, Q rides the
    vector HWDGE queue (first-needed chunks first so the first QK
    starts right after the framework preamble). Out-DMAs on sync.
All matmuls bf16 with f32 PSUM accumulation; softmax math in f32.
"""

import numpy as np
import ml_dtypes

B = 2
S = 2048
H = 32
KVH = 8
D = 128
HQ = H // KVH  # q heads per core = 4
W = 1024  # sliding window
SCALE = 0.08838834764831845
N_CORES = 8
BS = B * S  # 4096
NT = S // 128  # 16 k-tiles / q-tiles per sequence
NG = S // 512  # 4 q-blocks per sequence
VW = D + 1  # 129: V width with ones column
_BPG = 2  # 512-col bins per score group (3 banks x 2 slots, or 2 x 3)
GW = 512 * _BPG  # score-group width
_ST_BUFS = 6 // _BPG  # double/triple-buffered score groups = 6 banks

_BF16 = ml_dtypes.bfloat16

_CACHE = {}


def _groups_for_bh():
    """Static schedule: the band segments of one (b, h), packed into nine
    [128, 1536] score groups (3 bins of 512; every bin exactly full).

    Returns a list of 9 groups; each group is a list of segments
    (g, j, qv, n, off) laid out contiguously at column offset `off`.
    Within a group segments are ordered g-monotone (all of q-block g
    before any of g+1) so the PV accumulator pool (2 slots, 2 tiles per
    g) never holds more than two q-blocks alive. No segment's matmul
    output crosses a 512-col PSUM bank boundary.
    """
    per_g = []
    for g in range(NG):
        q0 = 512 * g
        fulls, ramps = [], []
        for j in range(max(0, 4 * g - 8), 4 * g + 4):
            qv = max(q0, 128 * j)
            qe = min(q0 + 512, 128 * j + 128 + W)
            seg = (g, j, qv, qe - qv)
            (fulls if qe - qv == 512 else ramps).append(seg)
        ramps.sort(key=lambda s: -s[3])
        per_g.append((fulls, ramps))

    bins = []  # list of lists of segs; every bin sums to 512
    open_bin = None  # partially-filled bin awaiting next-g exact fill
    for g in range(NG):
        fulls, ramps = per_g[g]
        ramps = list(ramps)
        if open_bin is not None:
            rem = 512 - sum(s[3] for s in open_bin)
            fit = next((s for s in ramps if s[3] == rem), None)
            assert fit is not None, (g, rem)
            open_bin.append(fit)
            ramps.remove(fit)
            bins.append(open_bin)
            open_bin = None
        gbins = [[f] for f in fulls]
        rbins = []
        for s in ramps:
            placed = False
            for rb in rbins:
                if sum(x[3] for x in rb) + s[3] <= 512:
                    rb.append(s)
                    placed = True
                    break
            if not placed:
                rbins.append([s])
        # keep at most one non-full bin open for the next g
        for rb in rbins:
            if sum(x[3] for x in rb) == 512:
                gbins.append(rb)
            else:
                assert open_bin is None, (g, rbins)
                open_bin = rb
        # arrange [full, full, ramp-bin] triples where possible: pair the
        # full bins two-per-group with one combo bin (helps a later fp8
        # DoubleRow pairing and spreads mask work evenly)
        fb = [b_ for b_ in gbins if len(b_) == 1]
        cb = [b_ for b_ in gbins if len(b_) > 1]
        ordered = []
        while fb or cb:
            take = fb[:2]
            fb = fb[2:]
            if cb and (len(take) < 3):
                take.append(cb.pop(0))
            while len(take) < 3 and fb:
                take.append(fb.pop(0))
            ordered.extend(take)
        bins.extend(ordered)
    assert open_bin is None
    assert len(bins) == 27 and all(sum(s[3] for s in b_) == 512 for b_ in bins)

    groups = []
    for k in range(0, 27, _BPG):
        segs = []
        for bi, b_ in enumerate(bins[k:k + _BPG]):
            off = 512 * bi
            for (g, j, qv, n) in b_:
                segs.append((g, j, qv, n, off))
                off += n
        segs.sort(key=lambda s: (s[0], s[4]))  # g-monotone
        gs = {s[0] for s in segs}
        assert len(gs) <= 2 and max(gs) - min(gs) <= 1, segs
        groups.append(segs)
    # sanity: every (g, j) tile appears exactly once with the right span
    seen = {}
    for segs in groups:
        for (g, j, qv, n, off) in segs:
            assert (g, j) not in seen
            seen[(g, j)] = (qv, n)
    assert sum(n for (qv, n) in seen.values()) == 27 * 512
    return groups


_GROUPS = _groups_for_bh()
_NPV = [[min(4 * g + s, 8) + 1 for s in range(4)] for g in range(NG)]


def _build_nc(opts=None):
    """Build + compile the single-core Bass/Tile program (SPMD across 8)."""
    from contextlib import ExitStack

    import concourse.bass as bass
    import concourse.tile as tile
    from concourse import bacc, mybir

    opts = dict(opts or {})
    fp32 = mybir.dt.float32
    bf16 = mybir.dt.bfloat16

    nc = bacc.Bacc("TRN2", target_bir_lowering=False, debug=False,
                   num_devices=N_CORES)

    qt_d = nc.dram_tensor("qt", [HQ, D, BS], bf16, kind="ExternalInput").ap()
    kt_d = nc.dram_tensor("kt", [D, BS], bf16, kind="ExternalInput").ap()
    vv_d = nc.dram_tensor("vv", [B, 128, NT * VW], bf16, kind="ExternalInput").ap()
    mk_d = nc.dram_tensor("mk", [128, 256], bf16, kind="ExternalInput").ap()
    out_d = nc.dram_tensor("out", [HQ, B, S, D], fp32, kind="ExternalOutput").ap()

    with tile.TileContext(nc) as tc, ExitStack() as ctx:
        mask_pool = ctx.enter_context(tc.tile_pool(name="mask", bufs=1))
        kt_pool = ctx.enter_context(tc.tile_pool(name="ktp", bufs=2))
        vv_pool = ctx.enter_context(tc.tile_pool(name="vvp", bufs=2))
        qt_pool = ctx.enter_context(tc.tile_pool(name="qtp", bufs=2))
        pt_pool = ctx.enter_context(tc.tile_pool(name="ptp",
                                                 bufs=opts.get("pt_bufs", 5)))
        osb_pool = ctx.enter_context(tc.tile_pool(name="osb", bufs=6))
        rec_pool = ctx.enter_context(tc.tile_pool(name="rec", bufs=6))
        st_pool = ctx.enter_context(
            tc.tile_pool(name="stp", bufs=opts.get("st_bufs", _ST_BUFS),
                         space="PSUM"))
        acc_pool = ctx.enter_context(
            tc.tile_pool(name="accp", bufs=2, space="PSUM"))

        masks = mask_pool.tile([128, 256], bf16)

        pools = (kt_pool, vv_pool, qt_pool, pt_pool, osb_pool, rec_pool,
                 st_pool, acc_pool)
        _body_once(nc, tc, mybir, masks, mk_d, *pools, qt_d, kt_d, vv_d,
                   out_d, opts)

    nc.compile()
    return nc


def _body_once(nc, tc, mybir, masks, mk_d, kt_pool, vv_pool, qt_pool,
               pt_pool, osb_pool, rec_pool, st_pool, acc_pool, qt_d, kt_d,
               vv_d, out_d, opts=None):
    opts = opts or {}
    fp32 = mybir.dt.float32
    bf16 = mybir.dt.bfloat16
    mask_eng = getattr(nc, opts.get("mask_eng", "gpsimd"))

    # prefetched input tiles, keyed (kind, b[, h]). All DMAs ride the
    # sync HWDGE queue (the only non-scalar HWDGE queue; the scalar
    # sequencer is the critical engine and a DMA trigger costs the
    # issuing sequencer ~670ns). Startup interleaves K/Q chunks
    # first-needed-first so the first QK starts right after the
    # framework preamble. Chunks are 1024-col (q/k) / 8-ktile (v)
    # aligned so every compute read lands inside a single chunk.
    kv_tiles = {}

    def load_kv(b, interleave_q=None):
        if ("k", b) in kv_tiles:
            return
        ktt = kt_pool.tile([128, S], bf16, name=f"ktt_{b}")
        vvt = vv_pool.tile([128, NT * VW], bf16, name=f"vvt_{b}")
        kv_tiles[("k", b)] = ktt
        kv_tiles[("v", b)] = vvt
        ksrc = kt_d[:, b * S:(b + 1) * S]
        qtt = None
        if interleave_q is not None:
            qtt = qt_pool.tile([128, S], bf16,
                               name=f"qtt_{b}_{interleave_q}")
            kv_tiles[("q", b, interleave_q)] = qtt
            qsrc = qt_d[interleave_q, :, b * S:(b + 1) * S]
        for c in range(2):
            nc.sync.dma_start(ktt[:, 1024 * c:1024 * (c + 1)],
                              ksrc[:, 1024 * c:1024 * (c + 1)])
            if qtt is not None:
                nc.sync.dma_start(qtt[:, 1024 * c:1024 * (c + 1)],
                                  qsrc[:, 1024 * c:1024 * (c + 1)])
        for c in range(2):
            nc.sync.dma_start(vvt[:, 8 * VW * c:8 * VW * (c + 1)],
                              vv_d[b][:, 8 * VW * c:8 * VW * (c + 1)])

    def load_q(b, h):
        if ("q", b, h) not in kv_tiles:
            qtt = qt_pool.tile([128, S], bf16, name=f"qtt_{b}_{h}")
            qsrc = qt_d[h, :, b * S:(b + 1) * S]
            for c in range(2):
                nc.sync.dma_start(qtt[:, 1024 * c:1024 * (c + 1)],
                                  qsrc[:, 1024 * c:1024 * (c + 1)])
            kv_tiles[("q", b, h)] = qtt

    load_kv(0, interleave_q=0)
    # masks aren't needed until the first post-exp multiply (~14us in);
    # load them behind the first K/Q/V chunks so they never delay QK
    nc.sync.dma_start(masks[:], mk_d[:])

    # flat schedule of all score groups across (b, h) so the software
    # pipeline (QK of group w ahead of exp of w-1 ahead of PV of w-2)
    # crosses head/batch boundaries without draining
    sched = []
    for b in range(B):
        for h in range(HQ):
            for gi in range(len(_GROUPS)):
                sched.append((b, h, gi))

    gctx = {}  # (b, h, g) -> accumulators and counters
    stt = {}   # w -> st tile
    ptt = {}   # w -> pt tile

    def emit_qk(w):
        b, h, gi = sched[w]
        if gi == 0:
            # prefetch next head's Q (or next batch's K/V/Q)
            if h + 1 < HQ:
                load_q(b, h + 1)
            elif b + 1 < B:
                load_kv(b + 1, interleave_q=0)
        ktt = kv_tiles[("k", b)]
        qtt = kv_tiles[("q", b, h)]
        st = st_pool.tile([128, GW], fp32, tag="st", name=f"st_{w}")
        for (g, j, qv, n, off) in _GROUPS[gi]:
            nc.tensor.matmul(
                st[:, off:off + n],
                ktt[:, 128 * j:128 * j + 128],
                qtt[:, qv:qv + n],
                start=True, stop=True,
            )
        stt[w] = st

    def front(w):
        # exp + masks for group w (diag masks on GpSimd, edge masks on
        # DVE — splitting them keeps either queue short enough that the
        # PV matmuls never stall on a pending mask)
        b, h, gi = sched[w]
        st = stt.pop(w)
        pt = pt_pool.tile([128, GW], bf16, tag="pt", name=f"pt_{w}")
        ptt[w] = pt
        width = max(off + n for (g, j, qv, n, off) in _GROUPS[gi])
        nc.scalar.activation(pt[:, 0:width], st[:, 0:width],
                             mybir.ActivationFunctionType.Exp, scale=SCALE)
        for (g, j, qv, n, off) in _GROUPS[gi]:
            if j >= 4 * g:
                # causal diagonal tile: first 128 cols of seg
                mask_eng.tensor_mul(
                    pt[:, off:off + 128], pt[:, off:off + 128],
                    masks[:, 0:128])
            if qv + n == 128 * j + 128 + W:
                # window edge tile: last 128 cols of seg
                nc.vector.tensor_mul(
                    pt[:, off + n - 128:off + n],
                    pt[:, off + n - 128:off + n],
                    masks[:, 128:256])

    def _pv_ops(gi):
        # PV matmul list for a group, g-monotone, mask-dependent ops
        # last within each g (gives the mask engines the whole QK window
        # to finish before the PE reaches the masked slices)
        ops = []
        for idx, (g, j, qv, n, off) in enumerate(_GROUPS[gi]):
            diag_i = j if j >= 4 * g else -1
            edge_i = j + 8 if qv + n == 128 * j + 128 + W else -1
            for i in range(max(4 * g, j), min(4 * g + 3, j + 8) + 1):
                masked = i == diag_i or i == edge_i
                ops.append((g, masked, idx, j, qv, off, i))
        ops.sort(key=lambda o: o[:3])
        return ops

    _SPLIT = opts.get("pv_split", 4)  # trailing MMs deferred one stage

    def back(w, part):
        # PV accumulation + finalizers for group w. part 0 = leading
        # MMs (emitted after QK(w+2)); part 1 = trailing MMs (emitted
        # BEFORE QK(w+3) as ready filler so the PE never idles - an
        # idle PE drops out of its max p-state and runs 2x slower).
        b, h, gi = sched[w]
        pt = ptt[w] if part == 0 else ptt.pop(w)
        vvt = kv_tiles[("v", b)]
        ops = _pv_ops(gi)
        ops = ops[:-_SPLIT] if part == 0 else ops[-_SPLIT:]
        for (g, masked, idx, j, qv, off, i) in ops:
            key = (b, h, g)
            ctx = gctx.get(key)
            if ctx is None:
                # two accumulators share each PSUM bank as ONE
                # accumulation group (see module docstring)
                ctx = gctx[key] = {
                    "acc": [acc_pool.tile([128, 2 * VW], fp32, tag="acc",
                                          name=f"acc_{b}_{h}_{g}_{y}")
                            for y in range(2)],
                    "bankpv": [0, 0],
                    "cpv": [0, 0, 0, 0],
                    "ot": osb_pool.tile([128, 512], fp32, tag="ot",
                                        name=f"ot_{b}_{h}_{g}"),
                    "fin": 0,
                }
            acc = ctx["acc"]
            cpv = ctx["cpv"]
            bankpv = ctx["bankpv"]
            npv = _NPV[g]
            banktot = [npv[0] + npv[1], npv[2] + npv[3]]
            s_ = i - 4 * g
            y, c0 = s_ // 2, (s_ % 2) * VW
            po = off + 128 * i - qv
            nc.tensor.matmul(
                acc[y][:, c0:c0 + VW],
                pt[:, po:po + 128],
                vvt[:, VW * j:VW * j + VW],
                start=(bankpv[y] == 0),
                stop=(bankpv[y] == banktot[y] - 1),
                skip_group_check=True,
            )
            bankpv[y] += 1
            cpv[s_] += 1
            if cpv[s_] == npv[s_]:
                src = acc[y]
                rec = rec_pool.tile([128, 1], fp32)
                nc.vector.reciprocal(rec[:], src[:, c0 + 128:c0 + 129])
                nc.vector.tensor_scalar_mul(
                    ctx["ot"][:, 128 * s_:128 * s_ + 128],
                    src[:, c0:c0 + 128], rec[:])
                ctx["fin"] += 1
                if ctx["fin"] == 4:
                    # one batched out-DMA per q-block.
                    # dst [p, t, d] iteration matches src [p, (t d)].
                    dst = out_d[h, b].rearrange(
                        "(t p) d -> p t d", p=128)[:, 4 * g:4 * g + 4, :]
                    nc.sync.dma_start(dst, ctx["ot"][:, :])
                    del gctx[(b, h, g)]

    # 4-stage software pipeline:
    #   PVtail(w-3) | QK(w) | exp+mask(w-1) | PVhead+finalize(w-2)
    # PE stream order per iteration is [PVtail(w-3), QK(w), PVhead(w-2)]:
    # PVtail is ready work (its masks completed an iteration ago) that
    # keeps the PE busy while exp(w-2) drains, so the in-order PE never
    # idles at QK(w)'s wait and stays in its max p-state.
    nsched = len(sched)
    for w in range(nsched + 3):
        if 3 <= w:
            back(w - 3, 1)
        if w < nsched:
            emit_qk(w)
        if 1 <= w < nsched + 1:
            front(w - 1)
        if 2 <= w < nsched + 2:
            back(w - 2, 0)


def _mask_np():
    """[128, 256] bf16: cols 0:128 diag keep r<=c; cols 128:256 edge keep c<r."""
    r = np.arange(128)[:, None]
    c = np.arange(128)[None, :]
    diag = (r <= c).astype(np.float32)
    edge = (c < r).astype(np.float32)
    return np.concatenate([diag, edge], axis=1).astype(_BF16)


def _prep_in_maps(query, key, value):
    q = np.asarray(query, dtype=np.float32).reshape(B, S, H, D)
    k = np.asarray(key, dtype=np.float32).reshape(B, S, KVH, D)
    v = np.asarray(value, dtype=np.float32).reshape(B, S, KVH, D)

    # [H, D, B*S] / [KVH, D, B*S]
    qt_all = np.ascontiguousarray(q.transpose(2, 3, 0, 1).reshape(H, D, BS)).astype(_BF16)
    kt_all = np.ascontiguousarray(k.transpose(2, 3, 0, 1).reshape(KVH, D, BS)).astype(_BF16)

    # V with ones column, packed [KVH, B, 128p, NT*VW] so that
    # vv[c, b, p, t*VW + d] = V'[b, 128t + p, c, d]
    vpad = np.concatenate([v, np.ones((B, S, KVH, 1), np.float32)], axis=3)
    vv_all = np.ascontiguousarray(
        vpad.reshape(B, NT, 128, KVH, VW).transpose(3, 0, 2, 1, 4)
        .reshape(KVH, B, 128, NT * VW)).astype(_BF16)

    mk = _mask_np()
    return [
        {
            "qt": np.ascontiguousarray(qt_all[HQ * c:HQ * c + HQ]),
            "kt": np.ascontiguousarray(kt_all[c]),
            "vv": np.ascontiguousarray(vv_all[c]),
            "mk": mk,
        }
        for c in range(N_CORES)
    ]


def _assemble(results):
    # results[c]["out"]: [HQ, B, S, D] -> full [B, S, H*D]
    o = np.stack([np.asarray(results[c]["out"], dtype=np.float32)
                  for c in range(N_CORES)])  # [8, HQ, B, S, D]
    return np.ascontiguousarray(
        o.transpose(2, 3, 0, 1, 4).reshape(B, S, H * D))


def kernel(query, key, value):
    from concourse import bass_utils

    if "nc" not in _CACHE:
        _CACHE["nc"] = _build_nc()
    nc = _CACHE["nc"]
    in_maps = _prep_in_maps(query, key, value)
    res = bass_utils.run_bass_kernel_spmd(
        nc, in_maps, core_ids=list(range(N_CORES)))
    return _assemble(res.results)


# revision 26
# speedup vs baseline: 1.0059x; 1.0059x over previous
"""Trainium2 Bass kernel for GQA sliding-window causal attention.

Problem: B=2, S=2048, H=32 q-heads, KVH=8 kv-heads, D=128,
sliding window 1024, causal, scale 1/sqrt(128). f32 I/O.

Sharding (8 cores, pure tensor parallel, no collectives): core c gets
kv-head c and its query-head group [4c, 4c+4). Each core computes full
attention for its 4 q-heads over both batch elements; host concatenates
along the head dim.

Per-core algorithm (banded, no online softmax needed since scores are
O(1) and exp never overflows):
  - Q and K live in SBUF transposed: [d=128 partitions, s free].
  - Scores computed transposed, ST[k, q], per (k-tile, q-span) band
    segment. Segments are packed into NINE uniform [128, 1536] PSUM
    groups per (b, h) (3 banks each, 27 exactly-full 512-col bins, one
    group straddling the g0/g1 q-block boundary) so ONE Exp activation
    covers 1536 contiguous columns -> 9 activations per (b,h), 72 per
    core. The scalar (Activation) engine is the hard floor of this
    decomposition (1 col/cycle @ 1.2 GHz, 110592 cols/core = 92 us);
    every activation carries ~300 ns of fixed cost (PSUM access init +
    decode) and its exec queue depth is 0, so fewer, wider activations
    win. The scalar queue issues NO DMAs (a DMA trigger costs the
    issuing sequencer ~670 ns).
  - P = exp(SCALE * ST) on ScalarE (scale folded into the activation),
    written as bf16 to SBUF.
  - Causal-diagonal and window-edge tiles are masked AFTER exp by
    multiplying with 0/1 bf16 mask tiles on GpSimd (exact zeros).
    GpSimd is otherwise idle; DVE keeps only the finalizers.
  - PV: out[q, 0:129] += PT_slice.T @ V'_j where V' has a ones column
    appended -> col 128 accumulates the softmax denominator for free.
    TWO [128, 129] accumulators share each PSUM bank as ONE hardware
    accumulation group: start_tensor_calc on the bank's first matmul
    marks the whole 2 KB zero region pending-zero (per-address
    zero-or-accumulate on write), so the sibling accumulator's first
    start=False write still lands on logical zeros. This halves the
    accumulator bank footprint (2 banks for 4 accumulators), buying
    3-bank score groups: PSUM = 2 score groups (6 banks) + 2 banks.
  - Normalize: out = psum[:, :128] * reciprocal(psum[:, 128]) on DVE,
    into a per-q-block [128, 512] staging tile flushed by ONE batched
    out-DMA on the sync queue.
  - 4-stage flat software pipeline across ALL (b, h, group) work:
    PVtail(w-3) | QK(w) | exp+mask(w-1) | PVhead+finalize(w-2). The
    PE stream per iteration is [PVtail(w-3), QK(w), PVhead(w-2)]: the
    deferred PV tail is always-ready filler that keeps the in-order PE
    from idling at QK's wait on exp(w-2) (an idle PE drops out of its
    max p-state and runs 2x slower), and deferring each group's last 4
    PV matmuls (the masked and q-block-transition ones) also gives the
    DVE a full iteration to finish the finalize reads that gate a new
    q-block's first accumulation into a reused PSUM bank. No pipeline
    drain at group/head/batch boundaries.
  - Input DMAs are chunked; K/V ride the sync HWDGE queue, Q rides the
    vector HWDGE queue (first-needed chunks first so the first QK
    starts right after the framework preamble). Out-DMAs on sync.
All matmuls bf16 with f32 PSUM accumulation; softmax math in f32.
"""

import numpy as np
import ml_dtypes

B = 2
S = 2048
H = 32
KVH = 8
D = 128
HQ = H // KVH  # q heads per core = 4
W = 1024  # sliding window
SCALE = 0.08838834764831845
N_CORES = 8
BS = B * S  # 4096
NT = S // 128  # 16 k-tiles / q-tiles per sequence
NG = S // 512  # 4 q-blocks per sequence
VW = D + 1  # 129: V width with ones column
_BPG = 3  # 512-col bins per score group (3 banks x 2 slots, or 2 x 3)
GW = 512 * _BPG  # score-group width
_ST_BUFS = 6 // _BPG  # double/triple-buffered score groups = 6 banks

_BF16 = ml_dtypes.bfloat16

_CACHE = {}


def _groups_for_bh():
    """Static schedule: the band segments of one (b, h), packed into nine
    [128, 1536] score groups (3 bins of 512; every bin exactly full).

    Returns a list of 9 groups; each group is a list of segments
    (g, j, qv, n, off) laid out contiguously at column offset `off`.
    Within a group segments are ordered g-monotone (all of q-block g
    before any of g+1) so the PV accumulator pool (2 slots, 2 tiles per
    g) never holds more than two q-blocks alive. No segment's matmul
    output crosses a 512-col PSUM bank boundary.
    """
    per_g = []
    for g in range(NG):
        q0 = 512 * g
        fulls, ramps = [], []
        for j in range(max(0, 4 * g - 8), 4 * g + 4):
            qv = max(q0, 128 * j)
            qe = min(q0 + 512, 128 * j + 128 + W)
            seg = (g, j, qv, qe - qv)
            (fulls if qe - qv == 512 else ramps).append(seg)
        ramps.sort(key=lambda s: -s[3])
        per_g.append((fulls, ramps))

    bins = []  # list of lists of segs; every bin sums to 512
    open_bin = None  # partially-filled bin awaiting next-g exact fill
    for g in range(NG):
        fulls, ramps = per_g[g]
        ramps = list(ramps)
        if open_bin is not None:
            rem = 512 - sum(s[3] for s in open_bin)
            fit = next((s for s in ramps if s[3] == rem), None)
            assert fit is not None, (g, rem)
            open_bin.append(fit)
            ramps.remove(fit)
            bins.append(open_bin)
            open_bin = None
        gbins = [[f] for f in fulls]
        rbins = []
        for s in ramps:
            placed = False
            for rb in rbins:
                if sum(x[3] for x in rb) + s[3] <= 512:
                    rb.append(s)
                    placed = True
                    break
            if not placed:
                rbins.append([s])
        # keep at most one non-full bin open for the next g
        for rb in rbins:
            if sum(x[3] for x in rb) == 512:
                gbins.append(rb)
            else:
                assert open_bin is None, (g, rbins)
                open_bin = rb
        # arrange [full, full, ramp-bin] triples where possible: pair the
        # full bins two-per-group with one combo bin (helps a later fp8
        # DoubleRow pairing and spreads mask work evenly)
        fb = [b_ for b_ in gbins if len(b_) == 1]
        cb = [b_ for b_ in gbins if len(b_) > 1]
        ordered = []
        while fb or cb:
            take = fb[:2]
            fb = fb[2:]
            if cb and (len(take) < 3):
                take.append(cb.pop(0))
            while len(take) < 3 and fb:
                take.append(fb.pop(0))
            ordered.extend(take)
        bins.extend(ordered)
    assert open_bin is None
    assert len(bins) == 27 and all(sum(s[3] for s in b_) == 512 for b_ in bins)

    groups = []
    for k in range(0, 27, _BPG):
        segs = []
        for bi, b_ in enumerate(bins[k:k + _BPG]):
            off = 512 * bi
            for (g, j, qv, n) in b_:
                segs.append((g, j, qv, n, off))
                off += n
        segs.sort(key=lambda s: (s[0], s[4]))  # g-monotone
        gs = {s[0] for s in segs}
        assert len(gs) <= 2 and max(gs) - min(gs) <= 1, segs
        groups.append(segs)
    # sanity: every (g, j) tile appears exactly once with the right span
    seen = {}
    for segs in groups:
        for (g, j, qv, n, off) in segs:
            assert (g, j) not in seen
            seen[(g, j)] = (qv, n)
    assert sum(n for (qv, n) in seen.values()) == 27 * 512
    return groups


_GROUPS = _groups_for_bh()
_NPV = [[min(4 * g + s, 8) + 1 for s in range(4)] for g in range(NG)]


def _build_nc(opts=None):
    """Build + compile the single-core Bass/Tile program (SPMD across 8)."""
    from contextlib import ExitStack

    import concourse.bass as bass
    import concourse.tile as tile
    from concourse import bacc, mybir

    opts = dict(opts or {})
    fp32 = mybir.dt.float32
    bf16 = mybir.dt.bfloat16

    nc = bacc.Bacc("TRN2", target_bir_lowering=False, debug=False,
                   num_devices=N_CORES)

    qt_d = nc.dram_tensor("qt", [HQ, D, BS], bf16, kind="ExternalInput").ap()
    kt_d = nc.dram_tensor("kt", [D, BS], bf16, kind="ExternalInput").ap()
    vv_d = nc.dram_tensor("vv", [B, 128, NT * VW], bf16, kind="ExternalInput").ap()
    mk_d = nc.dram_tensor("mk", [128, 256], bf16, kind="ExternalInput").ap()
    out_d = nc.dram_tensor("out", [HQ, B, S, D], fp32, kind="ExternalOutput").ap()

    with tile.TileContext(nc) as tc, ExitStack() as ctx:
        mask_pool = ctx.enter_context(tc.tile_pool(name="mask", bufs=1))
        kt_pool = ctx.enter_context(tc.tile_pool(name="ktp", bufs=2))
        vv_pool = ctx.enter_context(tc.tile_pool(name="vvp", bufs=2))
        qt_pool = ctx.enter_context(tc.tile_pool(name="qtp", bufs=2))
        pt_pool = ctx.enter_context(tc.tile_pool(name="ptp",
                                                 bufs=opts.get("pt_bufs", 5)))
        osb_pool = ctx.enter_context(tc.tile_pool(name="osb", bufs=6))
        rec_pool = ctx.enter_context(tc.tile_pool(name="rec", bufs=6))
        st_pool = ctx.enter_context(
            tc.tile_pool(name="stp", bufs=opts.get("st_bufs", _ST_BUFS),
                         space="PSUM"))
        acc_pool = ctx.enter_context(
            tc.tile_pool(name="accp", bufs=2, space="PSUM"))

        masks = mask_pool.tile([128, 256], bf16)

        pools = (kt_pool, vv_pool, qt_pool, pt_pool, osb_pool, rec_pool,
                 st_pool, acc_pool)
        _body_once(nc, tc, mybir, masks, mk_d, *pools, qt_d, kt_d, vv_d,
                   out_d, opts)

    nc.compile()
    return nc


def _body_once(nc, tc, mybir, masks, mk_d, kt_pool, vv_pool, qt_pool,
               pt_pool, osb_pool, rec_pool, st_pool, acc_pool, qt_d, kt_d,
               vv_d, out_d, opts=None):
    opts = opts or {}
    fp32 = mybir.dt.float32
    bf16 = mybir.dt.bfloat16
    mask_eng = getattr(nc, opts.get("mask_eng", "gpsimd"))

    # prefetched input tiles, keyed (kind, b[, h]). All DMAs ride the
    # sync HWDGE queue (the only non-scalar HWDGE queue; the scalar
    # sequencer is the critical engine and a DMA trigger costs the
    # issuing sequencer ~670ns). Startup interleaves K/Q chunks
    # first-needed-first so the first QK starts right after the
    # framework preamble. Chunks are 1024-col (q/k) / 8-ktile (v)
    # aligned so every compute read lands inside a single chunk.
    kv_tiles = {}

    def load_kv(b, interleave_q=None):
        if ("k", b) in kv_tiles:
            return
        ktt = kt_pool.tile([128, S], bf16, name=f"ktt_{b}")
        vvt = vv_pool.tile([128, NT * VW], bf16, name=f"vvt_{b}")
        kv_tiles[("k", b)] = ktt
        kv_tiles[("v", b)] = vvt
        ksrc = kt_d[:, b * S:(b + 1) * S]
        qtt = None
        if interleave_q is not None:
            qtt = qt_pool.tile([128, S], bf16,
                               name=f"qtt_{b}_{interleave_q}")
            kv_tiles[("q", b, interleave_q)] = qtt
            qsrc = qt_d[interleave_q, :, b * S:(b + 1) * S]
        for c in range(2):
            nc.sync.dma_start(ktt[:, 1024 * c:1024 * (c + 1)],
                              ksrc[:, 1024 * c:1024 * (c + 1)])
            if qtt is not None:
                nc.sync.dma_start(qtt[:, 1024 * c:1024 * (c + 1)],
                                  qsrc[:, 1024 * c:1024 * (c + 1)])
        for c in range(2):
            nc.sync.dma_start(vvt[:, 8 * VW * c:8 * VW * (c + 1)],
                              vv_d[b][:, 8 * VW * c:8 * VW * (c + 1)])

    def load_q(b, h):
        if ("q", b, h) not in kv_tiles:
            qtt = qt_pool.tile([128, S], bf16, name=f"qtt_{b}_{h}")
            qsrc = qt_d[h, :, b * S:(b + 1) * S]
            for c in range(2):
                nc.sync.dma_start(qtt[:, 1024 * c:1024 * (c + 1)],
                                  qsrc[:, 1024 * c:1024 * (c + 1)])
            kv_tiles[("q", b, h)] = qtt

    load_kv(0, interleave_q=0)
    # masks aren't needed until the first post-exp multiply (~14us in);
    # load them behind the first K/Q/V chunks so they never delay QK
    nc.sync.dma_start(masks[:], mk_d[:])

    # flat schedule of all score groups across (b, h) so the software
    # pipeline (QK of group w ahead of exp of w-1 ahead of PV of w-2)
    # crosses head/batch boundaries without draining
    sched = []
    for b in range(B):
        for h in range(HQ):
            for gi in range(len(_GROUPS)):
                sched.append((b, h, gi))

    gctx = {}  # (b, h, g) -> accumulators and counters
    stt = {}   # w -> st tile
    ptt = {}   # w -> pt tile

    def emit_qk(w):
        b, h, gi = sched[w]
        if gi == 0:
            # prefetch next head's Q (or next batch's K/V/Q)
            if h + 1 < HQ:
                load_q(b, h + 1)
            elif b + 1 < B:
                load_kv(b + 1, interleave_q=0)
        ktt = kv_tiles[("k", b)]
        qtt = kv_tiles[("q", b, h)]
        st = st_pool.tile([128, GW], fp32, tag="st", name=f"st_{w}")
        for (g, j, qv, n, off) in _GROUPS[gi]:
            nc.tensor.matmul(
                st[:, off:off + n],
                ktt[:, 128 * j:128 * j + 128],
                qtt[:, qv:qv + n],
                start=True, stop=True,
            )
        stt[w] = st

    def front(w):
        # exp + masks for group w (diag masks on GpSimd, edge masks on
        # DVE — splitting them keeps either queue short enough that the
        # PV matmuls never stall on a pending mask)
        b, h, gi = sched[w]
        st = stt.pop(w)
        pt = pt_pool.tile([128, GW], bf16, tag="pt", name=f"pt_{w}")
        ptt[w] = pt
        width = max(off + n for (g, j, qv, n, off) in _GROUPS[gi])
        nc.scalar.activation(pt[:, 0:width], st[:, 0:width],
                             mybir.ActivationFunctionType.Exp, scale=SCALE)
        for (g, j, qv, n, off) in _GROUPS[gi]:
            if j >= 4 * g:
                # causal diagonal tile: first 128 cols of seg
                mask_eng.tensor_mul(
                    pt[:, off:off + 128], pt[:, off:off + 128],
                    masks[:, 0:128])
            if qv + n == 128 * j + 128 + W:
                # window edge tile: last 128 cols of seg
                nc.vector.tensor_mul(
                    pt[:, off + n - 128:off + n],
                    pt[:, off + n - 128:off + n],
                    masks[:, 128:256])

    def _pv_ops(gi):
        # PV matmul list for a group, g-monotone, mask-dependent ops
        # last within each g (gives the mask engines the whole QK window
        # to finish before the PE reaches the masked slices)
        ops = []
        for idx, (g, j, qv, n, off) in enumerate(_GROUPS[gi]):
            diag_i = j if j >= 4 * g else -1
            edge_i = j + 8 if qv + n == 128 * j + 128 + W else -1
            for i in range(max(4 * g, j), min(4 * g + 3, j + 8) + 1):
                masked = i == diag_i or i == edge_i
                ops.append((g, masked, idx, j, qv, off, i))
        ops.sort(key=lambda o: o[:3])
        return ops

    _SPLIT = opts.get("pv_split", 4)  # trailing MMs deferred one stage

    def back(w, part):
        # PV accumulation + finalizers for group w. part 0 = leading
        # MMs (emitted after QK(w+2)); part 1 = trailing MMs (emitted
        # BEFORE QK(w+3) as ready filler so the PE never idles - an
        # idle PE drops out of its max p-state and runs 2x slower).
        b, h, gi = sched[w]
        pt = ptt[w] if part == 0 else ptt.pop(w)
        vvt = kv_tiles[("v", b)]
        ops = _pv_ops(gi)
        ops = ops[:-_SPLIT] if part == 0 else ops[-_SPLIT:]
        for (g, masked, idx, j, qv, off, i) in ops:
            key = (b, h, g)
            ctx = gctx.get(key)
            if ctx is None:
                # two accumulators share each PSUM bank as ONE
                # accumulation group (see module docstring)
                ctx = gctx[key] = {
                    "acc": [acc_pool.tile([128, 2 * VW], fp32, tag="acc",
                                          name=f"acc_{b}_{h}_{g}_{y}")
                            for y in range(2)],
                    "bankpv": [0, 0],
                    "cpv": [0, 0, 0, 0],
                    "ot": osb_pool.tile([128, 512], fp32, tag="ot",
                                        name=f"ot_{b}_{h}_{g}"),
                    "fin": 0,
                }
            acc = ctx["acc"]
            cpv = ctx["cpv"]
            bankpv = ctx["bankpv"]
            npv = _NPV[g]
            banktot = [npv[0] + npv[1], npv[2] + npv[3]]
            s_ = i - 4 * g
            y, c0 = s_ // 2, (s_ % 2) * VW
            po = off + 128 * i - qv
            nc.tensor.matmul(
                acc[y][:, c0:c0 + VW],
                pt[:, po:po + 128],
                vvt[:, VW * j:VW * j + VW],
                start=(bankpv[y] == 0),
                stop=(bankpv[y] == banktot[y] - 1),
                skip_group_check=True,
            )
            bankpv[y] += 1
            cpv[s_] += 1
            if cpv[s_] == npv[s_]:
                src = acc[y]
                rec = rec_pool.tile([128, 1], fp32)
                nc.vector.reciprocal(rec[:], src[:, c0 + 128:c0 + 129])
                nc.vector.tensor_scalar_mul(
                    ctx["ot"][:, 128 * s_:128 * s_ + 128],
                    src[:, c0:c0 + 128], rec[:])
                ctx["fin"] += 1
                if ctx["fin"] == 4:
                    # one batched out-DMA per q-block.
                    # dst [p, t, d] iteration matches src [p, (t d)].
                    dst = out_d[h, b].rearrange(
                        "(t p) d -> p t d", p=128)[:, 4 * g:4 * g + 4, :]
                    nc.sync.dma_start(dst, ctx["ot"][:, :])
                    del gctx[(b, h, g)]

    # 4-stage software pipeline:
    #   PVtail(w-3) | QK(w) | exp+mask(w-1) | PVhead+finalize(w-2)
    # PE stream order per iteration is [PVtail(w-3), QK(w), PVhead(w-2)]:
    # PVtail is ready work (its masks completed an iteration ago) that
    # keeps the PE busy while exp(w-2) drains, so the in-order PE never
    # idles at QK(w)'s wait and stays in its max p-state.
    nsched = len(sched)
    for w in range(nsched + 3):
        if 3 <= w:
            back(w - 3, 1)
        if w < nsched:
            emit_qk(w)
        if 1 <= w < nsched + 1:
            front(w - 1)
        if 2 <= w < nsched + 2:
            back(w - 2, 0)


def _mask_np():
    """[128, 256] bf16: cols 0:128 diag keep r<=c; cols 128:256 edge keep c<r."""
    r = np.arange(128)[:, None]
    c = np.arange(128)[None, :]
    diag = (r <= c).astype(np.float32)
    edge = (c < r).astype(np.float32)
    return np.concatenate([diag, edge], axis=1).astype(_BF16)


def _prep_in_maps(query, key, value):
    q = np.asarray(query, dtype=np.float32).reshape(B, S, H, D)
    k = np.asarray(key, dtype=np.float32).reshape(B, S, KVH, D)
    v = np.asarray(value, dtype=np.float32).reshape(B, S, KVH, D)

    # [H, D, B*S] / [KVH, D, B*S]
    qt_all = np.ascontiguousarray(q.transpose(2, 3, 0, 1).reshape(H, D, BS)).astype(_BF16)
    kt_all = np.ascontiguousarray(k.transpose(2, 3, 0, 1).reshape(KVH, D, BS)).astype(_BF16)

    # V with ones column, packed [KVH, B, 128p, NT*VW] so that
    # vv[c, b, p, t*VW + d] = V'[b, 128t + p, c, d]
    vpad = np.concatenate([v, np.ones((B, S, KVH, 1), np.float32)], axis=3)
    vv_all = np.ascontiguousarray(
        vpad.reshape(B, NT, 128, KVH, VW).transpose(3, 0, 2, 1, 4)
        .reshape(KVH, B, 128, NT * VW)).astype(_BF16)

    mk = _mask_np()
    return [
        {
            "qt": np.ascontiguousarray(qt_all[HQ * c:HQ * c + HQ]),
            "kt": np.ascontiguousarray(kt_all[c]),
            "vv": np.ascontiguousarray(vv_all[c]),
            "mk": mk,
        }
        for c in range(N_CORES)
    ]


def _assemble(results):
    # results[c]["out"]: [HQ, B, S, D] -> full [B, S, H*D]
    o = np.stack([np.asarray(results[c]["out"], dtype=np.float32)
                  for c in range(N_CORES)])  # [8, HQ, B, S, D]
    return np.ascontiguousarray(
        o.transpose(2, 3, 0, 1, 4).reshape(B, S, H * D))


def kernel(query, key, value):
    from concourse import bass_utils

    if "nc" not in _CACHE:
        _CACHE["nc"] = _build_nc()
    nc = _CACHE["nc"]
    in_maps = _prep_in_maps(query, key, value)
    res = bass_utils.run_bass_kernel_spmd(
        nc, in_maps, core_ids=list(range(N_CORES)))
    return _assemble(res.results)


# revision 30
# speedup vs baseline: 1.0063x; 1.0005x over previous
"""Trainium2 Bass kernel for GQA sliding-window causal attention.

Problem: B=2, S=2048, H=32 q-heads, KVH=8 kv-heads, D=128,
sliding window 1024, causal, scale 1/sqrt(128). f32 I/O.

Sharding (8 cores, pure tensor parallel, no collectives): core c gets
kv-head c and its query-head group [4c, 4c+4). Each core computes full
attention for its 4 q-heads over both batch elements; host concatenates
along the head dim.

Per-core algorithm (banded, no online softmax needed since scores are
O(1) and exp never overflows):
  - Q and K live in SBUF transposed: [d=128 partitions, s free].
  - Scores computed transposed, ST[k, q], per (k-tile, q-span) band
    segment. Segments are packed into NINE uniform [128, 1536] PSUM
    groups per (b, h) (3 banks each, 27 exactly-full 512-col bins, one
    group straddling the g0/g1 q-block boundary) so ONE Exp activation
    covers 1536 contiguous columns -> 9 activations per (b,h), 72 per
    core. The scalar (Activation) engine is the hard floor of this
    decomposition (1 col/cycle @ 1.2 GHz, 110592 cols/core = 92 us);
    every activation carries ~300 ns of fixed cost (PSUM access init +
    decode) and its exec queue depth is 0, so fewer, wider activations
    win. The scalar queue issues NO DMAs (a DMA trigger costs the
    issuing sequencer ~670 ns).
  - P = exp(SCALE * ST) on ScalarE (scale folded into the activation),
    written as bf16 to SBUF.
  - Causal-diagonal and window-edge tiles are masked AFTER exp by
    multiplying with 0/1 bf16 mask tiles on GpSimd (exact zeros).
    GpSimd is otherwise idle; DVE keeps only the finalizers.
  - PV: out[q, 0:129] += PT_slice.T @ V'_j where V' has a ones column
    appended -> col 128 accumulates the softmax denominator for free.
    TWO [128, 129] accumulators share each PSUM bank as ONE hardware
    accumulation group: start_tensor_calc on the bank's first matmul
    marks the whole 2 KB zero region pending-zero (per-address
    zero-or-accumulate on write), so the sibling accumulator's first
    start=False write still lands on logical zeros. This halves the
    accumulator bank footprint (2 banks for 4 accumulators), buying
    3-bank score groups: PSUM = 2 score groups (6 banks) + 2 banks.
  - Normalize: out = psum[:, :128] * reciprocal(psum[:, 128]) on DVE,
    into a per-q-block [128, 512] staging tile flushed by ONE batched
    out-DMA on the sync queue.
  - 4-stage flat software pipeline across ALL (b, h, group) work:
    PVtail(w-3) | QK(w) | exp+mask(w-1) | PVhead+finalize(w-2). The
    PE stream per iteration is [PVtail(w-3), QK(w), PVhead(w-2)]: the
    deferred PV tail is always-ready filler that keeps the in-order PE
    from idling at QK's wait on exp(w-2) (an idle PE drops out of its
    max p-state and runs 2x slower), and deferring each group's last 4
    PV matmuls (the masked and q-block-transition ones) also gives the
    DVE a full iteration to finish the finalize reads that gate a new
    q-block's first accumulation into a reused PSUM bank. No pipeline
    drain at group/head/batch boundaries.
  - Input DMAs are chunked; K/V ride the sync HWDGE queue, Q rides the
    vector HWDGE queue (first-needed chunks first so the first QK
    starts right after the framework preamble). Out-DMAs on sync.
All matmuls bf16 with f32 PSUM accumulation; softmax math in f32.
"""

import numpy as np
import ml_dtypes

B = 2
S = 2048
H = 32
KVH = 8
D = 128
HQ = H // KVH  # q heads per core = 4
W = 1024  # sliding window
SCALE = 0.08838834764831845
N_CORES = 8
BS = B * S  # 4096
NT = S // 128  # 16 k-tiles / q-tiles per sequence
NG = S // 512  # 4 q-blocks per sequence
VW = D + 1  # 129: V width with ones column
_BPG = 3  # 512-col bins per score group (3 banks x 2 slots, or 2 x 3)
GW = 512 * _BPG  # score-group width
_ST_BUFS = 6 // _BPG  # double/triple-buffered score groups = 6 banks

_BF16 = ml_dtypes.bfloat16

_CACHE = {}


def _groups_for_bh():
    """Static schedule: the band segments of one (b, h), packed into nine
    [128, 1536] score groups (3 bins of 512; every bin exactly full).

    Returns a list of 9 groups; each group is a list of segments
    (g, j, qv, n, off) laid out contiguously at column offset `off`.
    Within a group segments are ordered g-monotone (all of q-block g
    before any of g+1) so the PV accumulator pool (2 slots, 2 tiles per
    g) never holds more than two q-blocks alive. No segment's matmul
    output crosses a 512-col PSUM bank boundary.
    """
    per_g = []
    for g in range(NG):
        q0 = 512 * g
        fulls, ramps = [], []
        for j in range(max(0, 4 * g - 8), 4 * g + 4):
            qv = max(q0, 128 * j)
            qe = min(q0 + 512, 128 * j + 128 + W)
            seg = (g, j, qv, qe - qv)
            (fulls if qe - qv == 512 else ramps).append(seg)
        ramps.sort(key=lambda s: -s[3])
        per_g.append((fulls, ramps))

    bins = []  # list of lists of segs; every bin sums to 512
    open_bin = None  # partially-filled bin awaiting next-g exact fill
    for g in range(NG):
        fulls, ramps = per_g[g]
        ramps = list(ramps)
        if open_bin is not None:
            rem = 512 - sum(s[3] for s in open_bin)
            fit = next((s for s in ramps if s[3] == rem), None)
            assert fit is not None, (g, rem)
            open_bin.append(fit)
            ramps.remove(fit)
            bins.append(open_bin)
            open_bin = None
        gbins = [[f] for f in fulls]
        rbins = []
        for s in ramps:
            placed = False
            for rb in rbins:
                if sum(x[3] for x in rb) + s[3] <= 512:
                    rb.append(s)
                    placed = True
                    break
            if not placed:
                rbins.append([s])
        # keep at most one non-full bin open for the next g
        for rb in rbins:
            if sum(x[3] for x in rb) == 512:
                gbins.append(rb)
            else:
                assert open_bin is None, (g, rbins)
                open_bin = rb
        # arrange [full, full, ramp-bin] triples where possible: pair the
        # full bins two-per-group with one combo bin (helps a later fp8
        # DoubleRow pairing and spreads mask work evenly)
        fb = [b_ for b_ in gbins if len(b_) == 1]
        cb = [b_ for b_ in gbins if len(b_) > 1]
        ordered = []
        while fb or cb:
            take = fb[:2]
            fb = fb[2:]
            if cb and (len(take) < 3):
                take.append(cb.pop(0))
            while len(take) < 3 and fb:
                take.append(fb.pop(0))
            ordered.extend(take)
        bins.extend(ordered)
    assert open_bin is None
    assert len(bins) == 27 and all(sum(s[3] for s in b_) == 512 for b_ in bins)

    groups = []
    for k in range(0, 27, _BPG):
        segs = []
        for bi, b_ in enumerate(bins[k:k + _BPG]):
            off = 512 * bi
            for (g, j, qv, n) in b_:
                segs.append((g, j, qv, n, off))
                off += n
        segs.sort(key=lambda s: (s[0], s[4]))  # g-monotone
        gs = {s[0] for s in segs}
        assert len(gs) <= 2 and max(gs) - min(gs) <= 1, segs
        groups.append(segs)
    # sanity: every (g, j) tile appears exactly once with the right span
    seen = {}
    for segs in groups:
        for (g, j, qv, n, off) in segs:
            assert (g, j) not in seen
            seen[(g, j)] = (qv, n)
    assert sum(n for (qv, n) in seen.values()) == 27 * 512
    return groups


_GROUPS = _groups_for_bh()
_NPV = [[min(4 * g + s, 8) + 1 for s in range(4)] for g in range(NG)]

# PV segment lists per group. A cross-q-block segment (the group-0 tail
# that OPENS the next q-block) is carried into the FOLLOWING group's PV
# list (flag 1 = read the previous group's PT tile): a new q-block's
# first accumulation into a reused PSUM bank carries a WAR dependency on
# the previous block's DVE finalize reads of that bank, and emitting it
# a group later turns a zero-slack back-to-back hazard into a full
# iteration of slack.
_PV_SEGS = [[] for _ in _GROUPS]
for _gi, _segs in enumerate(_GROUPS):
    _base = min(s[0] for s in _segs)
    for _s in _segs:
        if _s[0] == _base:
            _PV_SEGS[_gi].append((_s, 0))
        else:
            assert _gi + 1 < len(_GROUPS)
            _PV_SEGS[_gi + 1].append((_s, 1))


def _build_nc(opts=None):
    """Build + compile the single-core Bass/Tile program (SPMD across 8)."""
    from contextlib import ExitStack

    import concourse.bass as bass
    import concourse.tile as tile
    from concourse import bacc, mybir

    opts = dict(opts or {})
    fp32 = mybir.dt.float32
    bf16 = mybir.dt.bfloat16

    nc = bacc.Bacc("TRN2", target_bir_lowering=False, debug=False,
                   num_devices=N_CORES)

    qt_d = nc.dram_tensor("qt", [HQ, D, BS], bf16, kind="ExternalInput").ap()
    kt_d = nc.dram_tensor("kt", [D, BS], bf16, kind="ExternalInput").ap()
    vv_d = nc.dram_tensor("vv", [B, 128, NT * VW], bf16, kind="ExternalInput").ap()
    mk_d = nc.dram_tensor("mk", [128, 256], bf16, kind="ExternalInput").ap()
    out_d = nc.dram_tensor("out", [HQ, B, S, D], fp32, kind="ExternalOutput").ap()

    with tile.TileContext(nc) as tc, ExitStack() as ctx:
        mask_pool = ctx.enter_context(tc.tile_pool(name="mask", bufs=1))
        kt_pool = ctx.enter_context(tc.tile_pool(name="ktp", bufs=2))
        vv_pool = ctx.enter_context(tc.tile_pool(name="vvp", bufs=2))
        qt_pool = ctx.enter_context(tc.tile_pool(name="qtp", bufs=2))
        pt_pool = ctx.enter_context(tc.tile_pool(name="ptp",
                                                 bufs=opts.get("pt_bufs", 6)))
        osb_pool = ctx.enter_context(tc.tile_pool(name="osb", bufs=6))
        rec_pool = ctx.enter_context(tc.tile_pool(name="rec", bufs=6))
        st_pool = ctx.enter_context(
            tc.tile_pool(name="stp", bufs=opts.get("st_bufs", _ST_BUFS),
                         space="PSUM"))
        acc_pool = ctx.enter_context(
            tc.tile_pool(name="accp", bufs=2, space="PSUM"))

        masks = mask_pool.tile([128, 256], bf16)

        pools = (kt_pool, vv_pool, qt_pool, pt_pool, osb_pool, rec_pool,
                 st_pool, acc_pool)
        _body_once(nc, tc, mybir, masks, mk_d, *pools, qt_d, kt_d, vv_d,
                   out_d, opts)

    nc.compile()
    return nc


def _body_once(nc, tc, mybir, masks, mk_d, kt_pool, vv_pool, qt_pool,
               pt_pool, osb_pool, rec_pool, st_pool, acc_pool, qt_d, kt_d,
               vv_d, out_d, opts=None):
    opts = opts or {}
    fp32 = mybir.dt.float32
    bf16 = mybir.dt.bfloat16
    mask_eng = getattr(nc, opts.get("mask_eng", "gpsimd"))

    # prefetched input tiles, keyed (kind, b[, h]). All DMAs ride the
    # sync HWDGE queue (the only non-scalar HWDGE queue; the scalar
    # sequencer is the critical engine and a DMA trigger costs the
    # issuing sequencer ~670ns). Startup interleaves K/Q chunks
    # first-needed-first so the first QK starts right after the
    # framework preamble. Chunks are 1024-col (q/k) / 8-ktile (v)
    # aligned so every compute read lands inside a single chunk.
    kv_tiles = {}

    def load_kv(b, interleave_q=None):
        if ("k", b) in kv_tiles:
            return
        ktt = kt_pool.tile([128, S], bf16, name=f"ktt_{b}")
        vvt = vv_pool.tile([128, NT * VW], bf16, name=f"vvt_{b}")
        kv_tiles[("k", b)] = ktt
        kv_tiles[("v", b)] = vvt
        ksrc = kt_d[:, b * S:(b + 1) * S]
        qtt = None
        if interleave_q is not None:
            qtt = qt_pool.tile([128, S], bf16,
                               name=f"qtt_{b}_{interleave_q}")
            kv_tiles[("q", b, interleave_q)] = qtt
            qsrc = qt_d[interleave_q, :, b * S:(b + 1) * S]
        for c in range(2):
            nc.sync.dma_start(ktt[:, 1024 * c:1024 * (c + 1)],
                              ksrc[:, 1024 * c:1024 * (c + 1)])
            if qtt is not None:
                nc.sync.dma_start(qtt[:, 1024 * c:1024 * (c + 1)],
                                  qsrc[:, 1024 * c:1024 * (c + 1)])
        for c in range(2):
            nc.sync.dma_start(vvt[:, 8 * VW * c:8 * VW * (c + 1)],
                              vv_d[b][:, 8 * VW * c:8 * VW * (c + 1)])

    def load_q(b, h):
        if ("q", b, h) not in kv_tiles:
            qtt = qt_pool.tile([128, S], bf16, name=f"qtt_{b}_{h}")
            qsrc = qt_d[h, :, b * S:(b + 1) * S]
            for c in range(2):
                nc.sync.dma_start(qtt[:, 1024 * c:1024 * (c + 1)],
                                  qsrc[:, 1024 * c:1024 * (c + 1)])
            kv_tiles[("q", b, h)] = qtt

    load_kv(0, interleave_q=0)
    # masks aren't needed until the first post-exp multiply (~14us in);
    # load them behind the first K/Q/V chunks so they never delay QK
    nc.sync.dma_start(masks[:], mk_d[:])

    # flat schedule of all score groups across (b, h) so the software
    # pipeline (QK of group w ahead of exp of w-1 ahead of PV of w-2)
    # crosses head/batch boundaries without draining
    sched = []
    for b in range(B):
        for h in range(HQ):
            for gi in range(len(_GROUPS)):
                sched.append((b, h, gi))

    gctx = {}  # (b, h, g) -> accumulators and counters
    stt = {}   # w -> st tile
    ptt = {}   # w -> pt tile

    def emit_qk(w):
        b, h, gi = sched[w]
        if gi == 0:
            # prefetch next head's Q (or next batch's K/V/Q)
            if h + 1 < HQ:
                load_q(b, h + 1)
            elif b + 1 < B:
                load_kv(b + 1, interleave_q=0)
        ktt = kv_tiles[("k", b)]
        qtt = kv_tiles[("q", b, h)]
        st = st_pool.tile([128, GW], fp32, tag="st", name=f"st_{w}")
        for (g, j, qv, n, off) in _GROUPS[gi]:
            nc.tensor.matmul(
                st[:, off:off + n],
                ktt[:, 128 * j:128 * j + 128],
                qtt[:, qv:qv + n],
                start=True, stop=True,
            )
        stt[w] = st

    def front(w):
        # exp + masks for group w (diag masks on GpSimd, edge masks on
        # DVE — splitting them keeps either queue short enough that the
        # PV matmuls never stall on a pending mask)
        b, h, gi = sched[w]
        st = stt.pop(w)
        pt = pt_pool.tile([128, GW], bf16, tag="pt", name=f"pt_{w}")
        ptt[w] = pt
        width = max(off + n for (g, j, qv, n, off) in _GROUPS[gi])
        nc.scalar.activation(pt[:, 0:width], st[:, 0:width],
                             mybir.ActivationFunctionType.Exp, scale=SCALE)
        for (g, j, qv, n, off) in _GROUPS[gi]:
            if j >= 4 * g:
                # causal diagonal tile: first 128 cols of seg
                mask_eng.tensor_mul(
                    pt[:, off:off + 128], pt[:, off:off + 128],
                    masks[:, 0:128])
            if qv + n == 128 * j + 128 + W:
                # window edge tile: last 128 cols of seg
                nc.vector.tensor_mul(
                    pt[:, off + n - 128:off + n],
                    pt[:, off + n - 128:off + n],
                    masks[:, 128:256])

    def _pv_ops(gi):
        # PV matmul list for a group: g-monotone, carried-in segs after
        # native ones, mask-dependent ops last within each class (gives
        # the mask engines the whole QK window to finish before the PE
        # reaches the masked slices)
        ops = []
        for idx, ((g, j, qv, n, off), carry) in enumerate(_PV_SEGS[gi]):
            diag_i = j if j >= 4 * g else -1
            edge_i = j + 8 if qv + n == 128 * j + 128 + W else -1
            for i in range(max(4 * g, j), min(4 * g + 3, j + 8) + 1):
                masked = i == diag_i or i == edge_i
                ops.append((g, carry, masked, idx, j, qv, off, i))
        ops.sort(key=lambda o: o[:4])
        return ops

    _SPLIT = opts.get("pv_split", 4)  # trailing MMs deferred one stage

    def back(w, part):
        # PV accumulation + finalizers for group w. part 0 = leading
        # MMs (emitted after QK(w+2)); part 1 = trailing MMs (emitted
        # BEFORE QK(w+3) as ready filler so the PE never idles - an
        # idle PE drops out of its max p-state and runs 2x slower).
        b, h, gi = sched[w]
        vvt = kv_tiles[("v", b)]
        ops = _pv_ops(gi)
        ops = ops[:-_SPLIT] if part == 0 else ops[-_SPLIT:]
        for (g, carry, masked, idx, j, qv, off, i) in ops:
            pt = ptt[w - 1] if carry else ptt[w]
            key = (b, h, g)
            ctx = gctx.get(key)
            if ctx is None:
                # two accumulators share each PSUM bank as ONE
                # accumulation group (see module docstring)
                ctx = gctx[key] = {
                    "acc": [acc_pool.tile([128, 2 * VW], fp32, tag="acc",
                                          name=f"acc_{b}_{h}_{g}_{y}")
                            for y in range(2)],
                    "bankpv": [0, 0],
                    "cpv": [0, 0, 0, 0],
                    "ot": osb_pool.tile([128, 512], fp32, tag="ot",
                                        name=f"ot_{b}_{h}_{g}"),
                    "fin": 0,
                }
            acc = ctx["acc"]
            cpv = ctx["cpv"]
            bankpv = ctx["bankpv"]
            npv = _NPV[g]
            banktot = [npv[0] + npv[1], npv[2] + npv[3]]
            s_ = i - 4 * g
            y, c0 = s_ // 2, (s_ % 2) * VW
            po = off + 128 * i - qv
            nc.tensor.matmul(
                acc[y][:, c0:c0 + VW],
                pt[:, po:po + 128],
                vvt[:, VW * j:VW * j + VW],
                start=(bankpv[y] == 0),
                stop=(bankpv[y] == banktot[y] - 1),
                skip_group_check=True,
            )
            bankpv[y] += 1
            cpv[s_] += 1
            if cpv[s_] == npv[s_]:
                src = acc[y]
                rec = rec_pool.tile([128, 1], fp32)
                nc.vector.reciprocal(rec[:], src[:, c0 + 128:c0 + 129])
                nc.vector.tensor_scalar_mul(
                    ctx["ot"][:, 128 * s_:128 * s_ + 128],
                    src[:, c0:c0 + 128], rec[:])
                ctx["fin"] += 1
                if ctx["fin"] == 4:
                    # one batched out-DMA per q-block.
                    # dst [p, t, d] iteration matches src [p, (t d)].
                    dst = out_d[h, b].rearrange(
                        "(t p) d -> p t d", p=128)[:, 4 * g:4 * g + 4, :]
                    nc.sync.dma_start(dst, ctx["ot"][:, :])
                    del gctx[(b, h, g)]

    # 4-stage software pipeline:
    #   PVtail(w-3) | QK(w) | exp+mask(w-1) | PVhead+finalize(w-2)
    # PE stream order per iteration is [PVtail(w-3), QK(w), PVhead(w-2)]:
    # PVtail is ready work (its masks completed an iteration ago) that
    # keeps the PE busy while exp(w-2) drains, so the in-order PE never
    # idles at QK(w)'s wait and stays in its max p-state.
    nsched = len(sched)
    for w in range(nsched + 3):
        if 3 <= w:
            back(w - 3, 1)
        if w < nsched:
            emit_qk(w)
        if 1 <= w < nsched + 1:
            front(w - 1)
        if 2 <= w < nsched + 2:
            back(w - 2, 0)


def _mask_np():
    """[128, 256] bf16: cols 0:128 diag keep r<=c; cols 128:256 edge keep c<r."""
    r = np.arange(128)[:, None]
    c = np.arange(128)[None, :]
    diag = (r <= c).astype(np.float32)
    edge = (c < r).astype(np.float32)
    return np.concatenate([diag, edge], axis=1).astype(_BF16)


def _prep_in_maps(query, key, value):
    q = np.asarray(query, dtype=np.float32).reshape(B, S, H, D)
    k = np.asarray(key, dtype=np.float32).reshape(B, S, KVH, D)
    v = np.asarray(value, dtype=np.float32).reshape(B, S, KVH, D)

    # [H, D, B*S] / [KVH, D, B*S]
    qt_all = np.ascontiguousarray(q.transpose(2, 3, 0, 1).reshape(H, D, BS)).astype(_BF16)
    kt_all = np.ascontiguousarray(k.transpose(2, 3, 0, 1).reshape(KVH, D, BS)).astype(_BF16)

    # V with ones column, packed [KVH, B, 128p, NT*VW] so that
    # vv[c, b, p, t*VW + d] = V'[b, 128t + p, c, d]
    vpad = np.concatenate([v, np.ones((B, S, KVH, 1), np.float32)], axis=3)
    vv_all = np.ascontiguousarray(
        vpad.reshape(B, NT, 128, KVH, VW).transpose(3, 0, 2, 1, 4)
        .reshape(KVH, B, 128, NT * VW)).astype(_BF16)

    mk = _mask_np()
    return [
        {
            "qt": np.ascontiguousarray(qt_all[HQ * c:HQ * c + HQ]),
            "kt": np.ascontiguousarray(kt_all[c]),
            "vv": np.ascontiguousarray(vv_all[c]),
            "mk": mk,
        }
        for c in range(N_CORES)
    ]


def _assemble(results):
    # results[c]["out"]: [HQ, B, S, D] -> full [B, S, H*D]
    o = np.stack([np.asarray(results[c]["out"], dtype=np.float32)
                  for c in range(N_CORES)])  # [8, HQ, B, S, D]
    return np.ascontiguousarray(
        o.transpose(2, 3, 0, 1, 4).reshape(B, S, H * D))


def kernel(query, key, value):
    from concourse import bass_utils

    if "nc" not in _CACHE:
        _CACHE["nc"] = _build_nc()
    nc = _CACHE["nc"]
    in_maps = _prep_in_maps(query, key, value)
    res = bass_utils.run_bass_kernel_spmd(
        nc, in_maps, core_ids=list(range(N_CORES)))
    return _assemble(res.results)
